# revision 1
# baseline (speedup 1.0000x reference)
"""CapsuleLayer dynamic-routing kernel for Trainium2 (8 NeuronCores).

Problem (hardcoded):
  inputs: [B=16, I=1152, Din=16] f32
  W:      [1, N=32, I=1152, D=64, Din=16] f32
  x_hat = einsum('nidk,bik->bnid', W[0], inputs)        # [B,N,I,D]
  3 routing iterations of per-(b,n,d) softmax over I (size-1-dim squash
  quirk makes everything elementwise in d), output [B,N,D,1] f32.

Key algebra used:
  * iter0: softmax(0) is uniform -> s0 = mean_i(x_hat).
  * b_t accumulates as x_hat * V_t with V_t = sum of past squash outputs,
    so neither b nor the logits are ever materialized.
  * softmax without max-subtraction is safe: |logit| <= ~50 in f32.

Mapping:
  * Shard N across 8 cores (4 capsules each); cores fully independent.
  * x_hat gen: contraction dim = (8 i's x 16 k) = 128.  Stationary = W slab
    [(ig,k)=128, (n2,d)=128]; moving = host-built block-diagonal input
    matrix [(ig,k)=128, (b,ig')=128].  Output tile [(n2,d), (b,ig)] per
    (i-block, capsule-pair), PSUM-accumulated duplicates give mean_i(x_hat).
  * Routing slice = (pair, b): ACT computes E=exp(V*x_hat) via per-partition
    scale AND denom=sum_i E via accum_out in one instruction; DVE computes
    P=E*x_hat (tensor_tensor, bf16 2x mode) then numer=sum_i P via
    tensor_scalar with accum_out (single-src 4x mode).  (TensorTensorReduce
    would fuse these but crashes this environment's runtime.)
  * squash's sqrt(s^2+eps) is computed as |s+1e-20| (ACT Abs) so the only
    ACT table funcs are Exp/Abs -- one table set, no reload thrashing.
"""

import numpy as np

# ---------------- problem constants (hardcoded per contract) ----------------
B, I, DIN = 16, 1152, 16
N, D = 32, 64
NCORES = 8
NL = N // NCORES        # 4 capsules per core
NPAIR = NL // 2         # 2 capsule-pairs per core (2 n's x 64 d = 128 parts)
IG = 8                  # i's folded into the contraction dim
NBLK = I // IG          # 144 i-blocks
CHUNK = 16              # i-blocks per DMA super-tile
NCHUNK = NBLK // CHUNK  # 9
EPS = 1e-9

_compiled = {}


def _build_program(stage="full", reps=1, NH=1):
    import concourse.bacc as bacc
    import concourse.mybir as mybir
    import concourse.tile as tile

    f32 = mybir.dt.float32
    bf16 = mybir.dt.bfloat16
    Alu = mybir.AluOpType
    Act = mybir.ActivationFunctionType

    nc = bacc.Bacc("TRN2", target_bir_lowering=False, debug=False)

    wslab_d = nc.declare_dram_parameter(
        "wslab", [NPAIR, NCHUNK, 128, CHUNK, 128], bf16, isOutput=False)
    inpblk_d = nc.declare_dram_parameter(
        "inpblk", [NCHUNK, 128, CHUNK, 128], bf16, isOutput=False)
    out_d = nc.declare_dram_parameter(
        "out", [NPAIR, 128, B], f32, isOutput=True)

    with tile.TileContext(nc) as tc:
        with (
            tc.tile_pool(name="wsup", bufs=3) as wpool,
            tc.tile_pool(name="isup", bufs=3) as ipool,
            tc.tile_pool(name="xbuf", bufs=1) as xpool,
            tc.tile_pool(name="escr", bufs=6) as epool,
            tc.tile_pool(name="pscr", bufs=6) as ppool,
            tc.tile_pool(name="small", bufs=3) as spool,
            tc.tile_pool(name="psum", bufs=3, space="PSUM") as psum,
            tc.tile_pool(name="psmean", bufs=1, space="PSUM") as psmean,
        ):
            X = [xpool.tile([128, NBLK, 128], bf16, tag=f"X{p}", name=f"X{p}")
                 for p in range(NPAIR)]

            epsb = xpool.tile([128, 1], f32, tag="epsb", name="epsb")
            nc.vector.memset(epsb[:], 1e-20)

            def squashW(s, out_ap, w):
                """out = s * s^2/((1+s^2) * sqrt(s^2+EPS)) on [128,w] f32.

                sqrt(s^2+eps) is approximated by |s + 1e-20| -- exact to
                f32 precision wherever the output is non-negligible, and
                keeps ACT on a single table set (Exp/Abs only, no Ln/Sqrt).
                """
                sq = spool.tile([128, w], f32, tag="sq")
                nc.vector.tensor_mul(sq[:], s, s)
                u = spool.tile([128, w], f32, tag="u")
                nc.vector.tensor_scalar_add(u[:], sq[:], 1.0)
                r = spool.tile([128, w], f32, tag="r")
                nc.vector.reciprocal(r[:], u[:])
                a = spool.tile([128, w], f32, tag="a")
                nc.scalar.activation(a[:], s, Act.Abs, bias=epsb[:])
                ra = spool.tile([128, w], f32, tag="ra")
                nc.vector.reciprocal(ra[:], a[:])
                t1 = spool.tile([128, w], f32, tag="t1")
                nc.vector.tensor_mul(t1[:], s, sq[:])
                t2 = spool.tile([128, w], f32, tag="t2")
                nc.vector.tensor_mul(t2[:], t1[:], r[:])
                nc.vector.tensor_mul(out_ap, t2[:], ra[:])

            import contextlib

            def rep_scope():
                if reps == 1:
                    return contextlib.nullcontext(0)
                return tc.For_i(0, reps, 1)

            with rep_scope():
              mean_l = {}
              for p in range(NPAIR):
                  # ---------------- generation of x_hat for this pair --------
                  mean_ps = psmean.tile([128, 128], f32, tag=f"mean{p}")
                  mean_l[p] = mean_ps
                  for c in range(NCHUNK):
                      wsup = wpool.tile([128, CHUNK, 128], bf16, tag="wsup")
                      nc.sync.dma_start(wsup[:], wslab_d[p, c])
                      isup = ipool.tile([128, CHUNK, 128], bf16, tag="isup")
                      nc.sync.dma_start(isup[:], inpblk_d[c])
                      for q in range(CHUNK // 4):
                          psx = psum.tile([128, 512], f32, tag=f"psx{p}")
                          for j in range(4):
                              cb = q * 4 + j
                              blk = c * CHUNK + cb
                              nc.tensor.matmul(
                                  psx[:, j * 128:(j + 1) * 128],
                                  wsup[:, cb, :], isup[:, cb, :],
                                  start=True, stop=True)
                              nc.tensor.matmul(
                                  mean_ps[:], wsup[:, cb, :], isup[:, cb, :],
                                  start=(blk == 0), stop=(blk == NBLK - 1))
                          blk0 = c * CHUNK + q * 4
                          if (c * 4 + q) % 3 == 2:
                              nc.scalar.copy(
                                  X[p][:, blk0:blk0 + 4, :], psx[:])
                          else:
                              nc.vector.tensor_copy(
                                  X[p][:, blk0:blk0 + 4, :], psx[:])

                  # ---------------- routing iteration 0 (uniform softmax) ----
                  mf = spool.tile([128, B], f32, tag="mf")
                  nc.vector.tensor_reduce(
                      mf[:], mean_ps[:].rearrange("p (b g) -> p b g", g=IG),
                      axis=mybir.AxisListType.X, op=Alu.add)
                  s0 = spool.tile([128, B], f32, tag="s0")
                  nc.vector.tensor_scalar_mul(s0[:], mf[:], 1.0 / I)
                  if stage == "gen":
                      nc.sync.dma_start(out_d[p], s0[:])
                      continue
                  V = spool.tile([128, B], f32, tag=f"V{p}0")
                  squashW(s0[:], V[:], B)              # V = v0
                  if stage == "squash":
                      nc.sync.dma_start(out_d[p], V[:])
                      continue

                  # ------------- routing iterations 1, 2 (b-half pipelined) --
                  for t in (1, 2):
                      denom = spool.tile([128, B], f32, tag=f"den{p}{t}")
                      numer = spool.tile([128, B], f32, tag=f"num{p}{t}")
                      newV = spool.tile([128, B], f32, tag=f"V{p}{t}")
                      for half in range(NH):
                          h0 = half * (B // NH)
                          h1 = h0 + B // NH
                          for b in range(h0, h1):
                              xv = X[p][:, :, b * IG:(b + 1) * IG]
                              E = epool.tile([128, NBLK, IG], bf16, tag="E")
                              nc.scalar.activation(
                                  E[:], xv, Act.Exp,
                                  scale=V[:, b:b + 1],
                                  accum_out=denom[:, b:b + 1])
                              if stage == "exp":
                                  continue
                              P = ppool.tile([128, NBLK, IG], bf16, tag="P")
                              nc.vector.tensor_mul(P[:], E[:], xv)
                              Q = ppool.tile([128, NBLK, IG], bf16, tag="Q")
                              nc.vector.tensor_scalar(
                                  out=Q[:], in0=P[:], scalar1=1.0, scalar2=None,
                                  op0=Alu.mult, op1=Alu.add,
                                  accum_out=numer[:, b:b + 1])
                          if stage == "exp":
                              continue
                          rd = spool.tile([128, B // NH], f32, tag="rd")
                          nc.vector.reciprocal(rd[:], denom[:, h0:h1])
                          st = spool.tile([128, B // NH], f32, tag="st")
                          nc.vector.tensor_mul(st[:], numer[:, h0:h1], rd[:])
                          if t < 2:
                              vh = spool.tile([128, B // NH], f32, tag="vh")
                              squashW(st[:], vh[:], B // NH)
                              nc.vector.tensor_add(
                                  newV[:, h0:h1], V[:, h0:h1], vh[:])
                          else:
                              squashW(st[:], newV[:, h0:h1], B // NH)
                      if stage == "exp":
                          nc.sync.dma_start(out_d[p], denom[:])
                          break
                      if t < 2:
                          V = newV
                      else:
                          nc.sync.dma_start(out_d[p], newV[:])

    nc.finalize()
    return nc


def _prep_host(inputs, W):
    """Build per-core W slabs, the shared block-diagonal input operand."""
    import ml_dtypes
    bf16 = ml_dtypes.bfloat16

    # wslab[core]: [NPAIR, NCHUNK, (ig,k)=128, cb=CHUNK, (n2,d)=128]
    wslabs = []
    W0 = W[0]  # [N, I, D, DIN]
    for core in range(NCORES):
        Wc = W0[core * NL:(core + 1) * NL]            # [4, I, D, DIN]
        a = Wc.reshape(NPAIR, 2, NCHUNK, CHUNK, IG, D, DIN)
        # axes: pair, n2, chunk, cb, ig, d, k -> pair, chunk, ig, k, cb, n2, d
        bmat = np.ascontiguousarray(a.transpose(0, 2, 4, 6, 3, 1, 5))
        wslabs.append(bmat.reshape(NPAIR, NCHUNK, 128, CHUNK, 128)
                      .astype(bf16))

    # inpblk: [NCHUNK, (ig,k)=128, cb=CHUNK, (b,ig')=128], block-diag in ig
    r = inputs.reshape(B, NCHUNK, CHUNK, IG, DIN).transpose(1, 2, 3, 0, 4)
    # r: [chunk, cb, ig', b, k]
    z = np.zeros((NCHUNK, IG, DIN, CHUNK, B, IG), dtype=np.float32)
    for g in range(IG):
        z[:, g, :, :, :, g] = r[:, :, g, :, :].transpose(0, 3, 1, 2)
    inpblk = z.reshape(NCHUNK, 128, CHUNK, 128).astype(bf16)
    return wslabs, inpblk


def kernel(inputs, W):
    from concourse.bass_utils import run_bass_kernel_spmd

    inputs = np.asarray(inputs, dtype=np.float32)
    W = np.asarray(W, dtype=np.float32)

    if "nc" not in _compiled:
        _compiled["nc"] = _build_program()
    nc = _compiled["nc"]

    wslabs, inpblk = _prep_host(inputs, W)
    in_maps = [{"wslab": wslabs[c], "inpblk": inpblk} for c in range(NCORES)]
    res = run_bass_kernel_spmd(nc, in_maps, list(range(NCORES))).results

    out = np.empty((B, N, D), dtype=np.float32)
    for c in range(NCORES):
        o = res[c]["out"]                       # [NPAIR, 128, B]
        o = o.reshape(NPAIR, 2, D, B).transpose(3, 0, 1, 2)  # [B,pair,n2,D]
        out[:, c * NL:(c + 1) * NL, :] = o.reshape(B, NL, D)
    return out[..., None]

